# revision 65
# baseline (speedup 1.0000x reference)
"""Trainium2 Bass kernel for nn_Attention_60979945668745.

Multi-head causal attention (B=2, S=2048, D=2048, H=32, hd=64) with
interleaved RoPE, sharded over 8 NeuronCores as DP2 (batch) x TP4 (heads).

Per-core computation (1 batch, 8 heads, feature slice F=512):
  Projections run in fp8e4m3 DoubleRow with hi/lo error compensation:
  operands are split w = wh + wl, x = xh + xl (both parts e4m3); each
  contraction d-pair is covered by 3 DoubleRow matmuls computing
  wh.xh + wl.xh + wh.xl (the lo.lo term is negligible), i.e. 0.75
  PE-cycles per 128-contraction column vs 1.0 for fp16.
  Attention (QK^T, exp, AV) and the output projection run in fp16.

  Per q-chunk c of 512 tokens the kernel emits: projections+RoPE for
  chunk c -> attention rows qc=c (needs only K/V chunks <= c, causal)
  -> output projection for chunk c.  The Tile scheduler overlaps the
  phases (projection matmuls fill PE while attention waits on exp).

RoPE: weight rows are permuted per head to [even dims | odd dims]; the
rotation reads the projection PSUM directly on DVE (a full-tile cos mul,
four 32-partition swapped sin muls -- PSUM sources are exempt from the
same-start-partition BIR rule -- and a Pool add).  The cos/sin tables
fold in the 1/(SW*SX) fp8 dequant scale.  Softmax sums come free via a
ones column appended to V (row 64 of the AV PSUM output); columns are
scaled by 1/sum (reciprocal + partition broadcast) at eviction.  exp
carries a constant bias (cancels after normalization) to keep fp16
ranges safe.
"""

import sys

for _p in ("/opt/trn_rl_repo", "/opt/pypackages"):
    if _p not in sys.path:
        sys.path.insert(0, _p)

import numpy as np
import ml_dtypes

import concourse.bacc as bacc
import concourse.mybir as mybir
from concourse.tile import TileContext
from concourse.bass_utils import run_bass_kernel_spmd

F32 = mybir.dt.float32
F16 = mybir.dt.float16
F8 = mybir.dt.float8e4
AF = mybir.ActivationFunctionType
DR = mybir.MatmulPerfMode.DoubleRow
E4M3 = ml_dtypes.float8_e4m3

DIM = 2048
N_HEADS = 32
HD = 64
BATCH = 2
SEQ = 2048
N_CORES = 8
DP = 2
TP = 4
H_LOC = N_HEADS // TP

EXP_BIAS = -7.0   # cancels in softmax; keeps unnormalized fp16 AV
                  # outputs (evicted before the 1/sum scaling) far from
                  # fp16 overflow
SW = 1024.0   # weight fp8 scale
SX = 16.0     # x fp8 scale


class Cfg:
    def __init__(self, T=SEQ, D=DIM, h_loc=H_LOC, tc=512):
        self.T = T
        self.D = D
        self.h_loc = h_loc
        self.F = h_loc * HD
        self.TC = tc
        self.n_ft = self.F // 128
        self.n_dt = D // 128
        self.n_tc = T // tc
        self.n_kt = T // 128
        self.R = tc // 128

    def key(self):
        return (self.T, self.D, self.h_loc, self.TC)


def build_nc(cfg: Cfg, block_kind, reps=1, dbg=False):
    """block_kind[kt][qc] in {'skip','full','tri','mask'} classifies the
    S^T block (k-tile kt) x (q-chunk qc)."""
    T, D, F, TC = cfg.T, cfg.D, cfg.F, cfg.TC
    n_ft, n_dt, n_tc, n_kt, R = cfg.n_ft, cfg.n_dt, cfg.n_tc, cfg.n_kt, cfg.R
    h_loc = cfg.h_loc

    nc = bacc.Bacc("TRN2", target_bir_lowering=False, debug=False,
                   num_devices=N_CORES)

    # fp8 hi/lo packed operands: rows = d, cols = [hi | lo] interleaved
    x8T = nc.dram_tensor("x8T", [D, 2 * T], F8, kind="ExternalInput")
    wq8 = nc.dram_tensor("wq8", [D, 2 * F], F8, kind="ExternalInput")
    wk8 = nc.dram_tensor("wk8", [D, 2 * F], F8, kind="ExternalInput")
    wv8 = nc.dram_tensor("wv8", [D, 2 * F], F8, kind="ExternalInput")
    woT = nc.dram_tensor("woT", [F, D], F16, kind="ExternalInput")
    cst_d = nc.dram_tensor("cst", [F, 2 * T], F16, kind="ExternalInput")
    tri_d = nc.dram_tensor("tri", [128, 128], F16, kind="ExternalInput")
    ones_d = nc.dram_tensor("ones", [128, 1], F16, kind="ExternalInput")
    nbias_d = nc.dram_tensor("nbias", [128, 1], F32, kind="ExternalInput")
    n_mask = sum(1 for kt in range(n_kt) for qc in range(n_tc)
                 if block_kind[kt][qc] == "mask")
    me_d = nc.dram_tensor("maskexp", [128, max(1, n_mask) * TC], F16,
                          kind="ExternalInput")
    mask_idx = {}
    mi = 0
    for kt in range(n_kt):
        for qc in range(n_tc):
            if block_kind[kt][qc] == "mask":
                mask_idx[(kt, qc)] = mi
                mi += 1

    y = nc.dram_tensor("y", [T, D], F16, kind="ExternalOutput")
    if dbg:
        qh_dump = nc.dram_tensor("qh_dump", [F, T], F16,
                                 kind="ExternalOutput")
        kh_dump = nc.dram_tensor("kh_dump", [F, T], F16,
                                 kind="ExternalOutput")
        v_dump = nc.dram_tensor("v_dump", [128, n_kt * h_loc * 65], F16,
                                kind="ExternalOutput")
        ot_dump = nc.dram_tensor("ot_dump", [F, T], F16,
                                 kind="ExternalOutput")
        rr_dump = nc.dram_tensor("rr_dump", [1, T], F32,
                                 kind="ExternalOutput")

    n_dg = 4                       # d-tiles per DMA/tile group
    n_g = n_dt // n_dg
    n_dc = D // TC

    with TileContext(nc) as tc_:
      with tc_.tile_pool(name="persist", bufs=1) as persist:
        # V_aug: per token-tile [128, h_loc*65]; col 64 of each slot = 1
        v_sb = persist.tile([128, n_kt * h_loc * 65], F16)
        tri_sb = persist.tile([128, 128], F16)
        onesc = persist.tile([128, 1], F16, tag="onesc")
        nbias = persist.tile([128, 1], F32, tag="nbias")
        # resident rope'd Q^T / K^T, feature-major fp16
        qhs = [persist.tile([128, T], F16, tag=f"qh{ft}", name=f"qh{ft}")
               for ft in range(n_ft)]
        khs = [persist.tile([128, T], F16, tag=f"kh{ft}", name=f"kh{ft}")
               for ft in range(n_ft)]
        # per-head attention output (feature-major, fp16, unnormalized
        # then scaled in place)
        ot_sb = persist.tile([128, n_ft * T], F16, tag="ot")
        wo_sb = persist.tile([128, n_ft * D], F16, tag="wo")

        with tc_.tile_pool(name="wr", bufs=1) as wpool, \
             tc_.tile_pool(name="x", bufs=n_g + 2) as xpool, \
             tc_.tile_pool(name="t", bufs=6) as tpool, \
             tc_.tile_pool(name="cs", bufs=3) as cspool, \
             tc_.tile_pool(name="es", bufs=4) as espool, \
             tc_.tile_pool(name="sc", bufs=3) as scpool, \
             tc_.tile_pool(name="ys", bufs=2) as ypool, \
             tc_.tile_pool(name="pj", bufs=2, space="PSUM") as pjpool, \
             tc_.tile_pool(name="pss", bufs=2, space="PSUM") as sspool, \
             tc_.tile_pool(name="po", bufs=2, space="PSUM") as popool:
            # fp8 weights resident: per group g a [128, n_dg*2*F] tile
            wq_ts = [wpool.tile([128, n_dg * 2 * F], F8, tag=f"wq{g}",
                                name=f"wq{g}") for g in range(n_g)]
            wk_ts = [wpool.tile([128, n_dg * 2 * F], F8, tag=f"wk{g}",
                                name=f"wk{g}") for g in range(n_g)]
            wv_ts = [wpool.tile([128, n_dg * 2 * F], F8, tag=f"wv{g}",
                                name=f"wv{g}") for g in range(n_g)]
            def load_w(wts, wdr):
                # group-major so group 0's matmuls can start immediately
                for g in range(n_g):
                    nc.sync.dma_start(
                        out=wts[g][:, :].rearrange(
                            "p (j c) -> p j c", j=n_dg),
                        in_=wdr[g * n_dg * 128:(g + 1) * n_dg * 128,
                                :].rearrange("(j p) c -> p j c", p=128))
            load_w(wq_ts, wq8)
            load_w(wk_ts, wk8)
            nc.sync.dma_start(out=tri_sb[:, :], in_=tri_d[:, :])
            nc.sync.dma_start(out=onesc[:, :], in_=ones_d[:, :])
            nc.sync.dma_start(out=nbias[:, :], in_=nbias_d[:, :])
            ones_view = v_sb[:, :].rearrange("p (n c) -> p n c",
                                             c=65)[:, :, 64]
            nc.vector.tensor_copy(
                ones_view, onesc[:, :].broadcast_to([128, n_kt * h_loc]))

            # ---------------- emission helpers -----------------------
            def emit_attn(qc, h_lo, h_hi):
                kinds = [block_kind[kt][qc] for kt in range(n_kt)]
                live = [kt for kt in range(n_kt) if kinds[kt] != "skip"]
                if not live:
                    return
                rs = rb = None
                for h in range(h_lo, h_hi):
                    po = (h * HD) % 128
                    ft_h = (h * HD) // 128
                    qh = qhs[ft_h][po:po + 64, :]
                    kh = khs[ft_h][po:po + 64, :]
                    osl = ot_sb[po:po + 64, ft_h * T + qc * TC:
                                ft_h * T + (qc + 1) * TC]
                    pso = popool.tile([128, TC], F32, tag="pso")
                    first = True
                    pairs = [live[i:i + 2] for i in range(0, len(live), 2)]
                    for pair in pairs:
                        ps = sspool.tile([128, 2 * TC], F32, tag="pss")
                        es = espool.tile([128, 2 * TC], F16, tag="es")
                        # valid block ih occupies ps cols
                        # [base(ih) : base(ih) + TC - off(ih)] (packed
                        # contiguously so exp covers no causal garbage)
                        offs, bases = [], []
                        pos = 0
                        for kt in pair:
                            off = (max(0, kt * 128 - qc * TC)
                                   if kinds[kt] == "tri" else 0)
                            offs.append(off)
                            bases.append(pos)
                            pos += TC - off
                        for ih, kt in enumerate(pair):
                            off, base = offs[ih], bases[ih]
                            nc.tensor.matmul(
                                ps[:, base:base + TC - off],
                                kh[:, kt * 128:(kt + 1) * 128],
                                qh[:, qc * TC + off:(qc + 1) * TC],
                                start=True, stop=True)
                        nc.scalar.activation(es[:, 0:pos], ps[:, 0:pos],
                                             AF.Exp, scale=0.125,
                                             bias=nbias[:, :])
                        for ih, kt in enumerate(pair):
                            off, base = offs[ih], bases[ih]
                            if kinds[kt] == "tri":
                                nc.vector.tensor_mul(
                                    es[:, base:base + 128],
                                    es[:, base:base + 128],
                                    tri_sb[:, :])
                            elif kinds[kt] == "mask":
                                mslab = mask_idx[(kt, qc)]
                                mt = espool.tile([128, TC], F16, tag="mt")
                                nc.sync.dma_start(
                                    out=mt[:, :],
                                    in_=me_d[:, mslab * TC:
                                             (mslab + 1) * TC])
                                nc.vector.tensor_mul(
                                    es[:, base:base + TC],
                                    es[:, base:base + TC],
                                    mt[:, :])
                            nc.tensor.matmul(
                                pso[0:65, off:TC],
                                v_sb[:, kt * h_loc * 65 + h * 65:
                                     kt * h_loc * 65 + h * 65 + 65],
                                es[:, base:base + TC - off],
                                start=first,
                                stop=(kt == live[-1]))
                            first = False
                    # normalize columns by 1/sums (row 64) at eviction
                    rs = scpool.tile([1, TC], F32, tag="rs")
                    nc.vector.tensor_copy(rs[:, :], pso[64:65, :])
                    rr = scpool.tile([1, TC], F32, tag="rr")
                    nc.vector.reciprocal_approx_fast(rr[:, :], rs[:, :])
                    rb = scpool.tile([64, TC], F32, tag="rb")
                    nc.gpsimd.partition_broadcast(rb[:, :], rr[:, :])
                    if dbg and h == 0 and qc == 0:
                        nc.sync.dma_start(
                            out=rr_dump[:, qc * TC:(qc + 1) * TC],
                            in_=rr[:, :])
                    nc.vector.tensor_mul(osl, pso[0:64, :], rb[:, :])

            def emit_p3(qc, evict_eng=None):
                if evict_eng is None:
                    def evict(out, in_):
                        nc.scalar.activation(out, in_, AF.Copy)
                else:
                    evict = evict_eng.tensor_copy
                for ttile in range(qc * R, (qc + 1) * R):
                    ysrow = ypool.tile([128, D], F16, tag="ysrow")
                    for dc in range(n_dc):
                        psy = pjpool.tile([128, TC], F32, tag="ps")
                        for fk in range(n_ft):
                            nc.tensor.matmul(
                                psy[:, :],
                                ot_sb[:, fk * T + ttile * 128:
                                      fk * T + (ttile + 1) * 128],
                                wo_sb[:, fk * D + dc * TC:
                                      fk * D + (dc + 1) * TC],
                                start=(fk == 0),
                                stop=(fk == n_ft - 1))
                        evict(ysrow[:, dc * TC:(dc + 1) * TC], psy[:, :])
                    nc.sync.dma_start(
                        out=y[ttile * 128:(ttile + 1) * 128, :],
                        in_=ysrow[:, :])

            for _rep in range(reps):
              for c in range(n_tc):
                cs = slice(c * TC, (c + 1) * TC)
                # ============ phase 1: projections + RoPE, chunk c =======
                xg = []
                for g in range(n_g):
                    xtile = xpool.tile([128, n_dg * 2 * TC], F8, tag="xt")
                    xin = x8T[g * n_dg * 128:(g + 1) * n_dg * 128,
                              :].rearrange("(j p) (two t) -> p j two t",
                                           p=128, two=2)
                    xout = xtile[:, :].rearrange(
                        "p (j two t) -> p j two t", j=n_dg, two=2)
                    for hl in range(2):
                        nc.scalar.dma_start(out=xout[:, :, hl, :],
                                            in_=xin[:, :, hl, cs])
                    xg.append(xtile)
                if c == 0:
                    load_w(wv_ts, wv8)
                    for fk in range(n_ft):
                        nc.sync.dma_start(
                            out=wo_sb[:, fk * D:(fk + 1) * D],
                            in_=woT[fk * 128:(fk + 1) * 128, :])

                def wv4(wts, g):
                    return wts[g][:, :].rearrange(
                        "p (j two f) -> p j two f", j=n_dg, two=2)

                def xv4(g):
                    return xg[g][:, :].rearrange(
                        "p (j two t) -> p j two t", j=n_dg, two=2)

                def emit_qk_group(wts, dest, ft, pool=None, ptag="ps"):
                    # Q/K feature-major with fused RoPE
                    fs = slice(ft * 128, (ft + 1) * 128)
                    ps = (pool or pjpool).tile([128, TC], F32, tag=ptag)
                    for dp in range(0, n_dt, 2):
                        g, j = divmod(dp, n_dg)
                        w4 = wv4(wts, g)
                        x4 = xv4(g)
                        wh = w4[:, j:j + 2, 0, fs]
                        wl = w4[:, j:j + 2, 1, fs]
                        xh = x4[:, j:j + 2, 0, :]
                        xl = x4[:, j:j + 2, 1, :]
                        nc.tensor.matmul(ps[:, :], wh, xh, perf_mode=DR,
                                         start=(dp == 0), stop=False)
                        nc.tensor.matmul(ps[:, :], wl, xh, perf_mode=DR,
                                         start=False, stop=False)
                        nc.tensor.matmul(ps[:, :], wh, xl, perf_mode=DR,
                                         start=False,
                                         stop=(dp == n_dt - 2))
                    # RoPE: dest[a] = ps[a]*ct[a] + ps[a^32]*st[a]
                    # (swapped operand reads PSUM -- partition-shifted
                    # SBUF operands fail BIR verification)
                    cst_sb = cspool.tile([128, 2 * TC], F16, tag="cst")
                    nc.scalar.dma_start(
                        out=cst_sb[:, :].rearrange(
                            "p (two t) -> p two t", two=2),
                        in_=cst_d[fs, :].rearrange(
                            "p (two t) -> p two t", two=2)[:, :, cs])
                    t1 = tpool.tile([128, TC], F16, tag="t1")
                    nc.vector.tensor_mul(t1[:, :], ps[:, :],
                                         cst_sb[:, 0:TC])
                    qs = tpool.tile([128, TC], F16, tag="qs")
                    for s in range(4):
                        a, b = s * 32, (s ^ 1) * 32
                        nc.vector.tensor_mul(
                            qs[a:a + 32, :], ps[b:b + 32, :],
                            cst_sb[a:a + 32, TC:2 * TC])
                    nc.gpsimd.tensor_add(
                        dest[ft][:, cs], t1[:, :], qs[:, :])

                def emit_v_group(tt):
                    # V token-major (stationary = x token slices)
                    kt = c * R + tt
                    psv = pjpool.tile([128, F], F32, tag="ps")
                    for dp in range(0, n_dt, 2):
                        g, j = divmod(dp, n_dg)
                        w4 = wv4(wv_ts, g)
                        x4 = xv4(g)
                        tsl = slice(tt * 128, (tt + 1) * 128)
                        xh = x4[:, j:j + 2, 0, tsl]
                        xl = x4[:, j:j + 2, 1, tsl]
                        wh = w4[:, j:j + 2, 0, :]
                        wl = w4[:, j:j + 2, 1, :]
                        nc.tensor.matmul(psv[:, :], xh, wh, perf_mode=DR,
                                         start=(dp == 0), stop=False)
                        nc.tensor.matmul(psv[:, :], xl, wh, perf_mode=DR,
                                         start=False, stop=False)
                        nc.tensor.matmul(psv[:, :], xh, wl, perf_mode=DR,
                                         start=False,
                                         stop=(dp == n_dt - 2))
                    base = kt * h_loc * 65
                    vout = v_sb[:, base:base + h_loc * 65]
                    vout = vout.rearrange(
                        "p (h c) -> p h c", c=65)[:, :, 0:64]
                    vin = psv[:, :].rearrange("p (h c) -> p h c", c=64)
                    nc.scalar.activation(vout, vin, AF.Copy,
                                         scale=1.0 / (SW * SX))

                for ft in range(n_ft):
                    emit_qk_group(wq_ts, qhs, ft)
                for ft in range(n_ft):
                    emit_qk_group(wk_ts, khs, ft)
                for tt in range(R):
                    emit_v_group(tt)

                # ---- attention staggered at half-row granularity: each
                # row's first half right after its chunk, second half one
                # chunk later; keeps exp (ACT) uniformly loaded while the
                # next chunk's projections fill the PE.  The output
                # projection (pure PE, no ACT) is all deferred to the end
                # where it fills the exp-wait gaps of the late rows.
                hh = h_loc // 2
                if c >= 1:
                    emit_attn(c - 1, hh, h_loc)
                emit_attn(c, 0, hh)
              emit_attn(n_tc - 1, hh, h_loc)
              for qc in range(n_tc):
                  emit_p3(qc, evict_eng=nc.vector if qc == n_tc - 1
                          else None)
              if dbg:
                nc.sync.dma_start(out=v_dump[:, :], in_=v_sb[:, :])
                for ft in range(n_ft):
                    nc.sync.dma_start(
                        out=qh_dump[ft * 128:(ft + 1) * 128, :],
                        in_=qhs[ft][:, :])
                    nc.sync.dma_start(
                        out=kh_dump[ft * 128:(ft + 1) * 128, :],
                        in_=khs[ft][:, :])
                    nc.sync.dma_start(
                        out=ot_dump[ft * 128:(ft + 1) * 128, :],
                        in_=ot_sb[:, ft * T:(ft + 1) * T])

    nc.compile()
    return nc


# ---------------- host-side preparation ----------------

def _rope_tables(T, start_pos, heads, scale):
    """Reference RoPE uses a PER-HEAD angle: theta = base[head] * pos.
    Returns [len(heads)*64, T] fp32 ct / st tables in the permuted
    [evens|odds] feature order, times `scale` (fp8 dequant fold).
    st is TARGET-indexed: even-half rows carry -s, odd-half +s
    (qs[a] = ps[a^32] * st[a])."""
    base = 1.0 / (10000.0 ** (np.arange(0, HD, 2, dtype=np.float64) / HD))
    pos = start_pos + np.arange(T, dtype=np.float64)
    F_ = len(heads) * HD
    ct = np.empty((F_, T), np.float32)
    st = np.empty((F_, T), np.float32)
    for i, g in enumerate(heads):
        th = base[g] * pos                        # [T]
        c, s = np.cos(th) * scale, np.sin(th) * scale
        b = i * HD
        ct[b:b + 64] = c[None, :]
        st[b:b + 32] = -s[None, :]
        st[b + 32:b + 64] = s[None, :]
    return ct, st


def _perm():
    p = np.empty(HD, np.int64)
    p[:32] = np.arange(0, HD, 2)
    p[32:] = np.arange(1, HD, 2)
    return p


def _hi_lo_pack(a, scale):
    """a: [rows, cols] fp32 -> [rows, 2*cols] e4m3 (hi | lo per row)."""
    s = (a * scale).astype(np.float32)
    hi = s.astype(E4M3)
    lo = (s - hi.astype(np.float32)).astype(E4M3)
    out = np.empty((a.shape[0], 2 * a.shape[1]), E4M3)
    out[:, :a.shape[1]] = hi
    out[:, a.shape[1]:] = lo
    return out


def _classify_mask(mask, T, TC):
    """mask: [S,S] additive (rows=q, cols=k). Returns block_kind[kt][qc] and
    packed exp-mask slabs ([k,q] orientation) for 'mask' blocks."""
    n_kt = T // 128
    n_tc = T // TC
    tri = (np.arange(TC)[None, :] >= np.arange(128)[:, None])
    kinds = [[None] * n_tc for _ in range(n_kt)]
    for kt in range(n_kt):
        for qc in range(n_tc):
            blk = mask[qc * TC:(qc + 1) * TC, kt * 128:(kt + 1) * 128].T
            if np.all(blk <= -1e8):
                kinds[kt][qc] = "skip"
            elif np.all(blk == 0.0):
                kinds[kt][qc] = "full"
            else:
                off = kt * 128 - qc * TC
                is_tri = False
                if 0 <= off <= TC - 128:
                    ref = np.full((128, TC), -1e9, np.float32)
                    ref[:, off:] = np.where(tri[:, :TC - off], 0.0, -1e9)
                    is_tri = bool(np.array_equal(blk, ref))
                kinds[kt][qc] = "tri" if is_tri else "mask"
    # a 'tri' block may not open an accumulation group at off>0
    for qc in range(n_tc):
        for kt in range(n_kt):
            k = kinds[kt][qc]
            if k == "skip":
                continue
            if k == "tri" and kt * 128 - qc * TC > 0:
                kinds[kt][qc] = "mask"
            break
    slabs = []
    for kt in range(n_kt):
        for qc in range(n_tc):
            if kinds[kt][qc] == "mask":
                blk = mask[qc * TC:(qc + 1) * TC,
                           kt * 128:(kt + 1) * 128].T
                slabs.append(np.exp(blk.astype(np.float64)
                                    ).astype(np.float32))
    me = (np.concatenate(slabs, axis=1) if slabs
          else np.zeros((128, TC), np.float32))
    return kinds, me


_CACHE = {}


def get_nc(cfg: Cfg, block_kind):
    key = (cfg.key(), tuple(tuple(r) for r in block_kind))
    if key not in _CACHE:
        _CACHE[key] = build_nc(cfg, block_kind)
    return _CACHE[key]


def prepare_in_maps(x, wq, wk, wv, wo, mask, start_pos, cfg):
    x = np.asarray(x, np.float32)
    wq = np.asarray(wq, np.float32)
    wk = np.asarray(wk, np.float32)
    wv = np.asarray(wv, np.float32)
    wo = np.asarray(wo, np.float32)
    mask2d = np.asarray(mask, np.float32).reshape(mask.shape[-2],
                                                  mask.shape[-1])
    sp = int(np.asarray(start_pos))

    tri01 = (np.arange(128)[None, :] >= np.arange(128)[:, None]
             ).astype(np.float16)
    kinds, me = _classify_mask(mask2d, cfg.T, cfg.TC)

    perm = _perm()
    x8_b = [_hi_lo_pack(np.ascontiguousarray(x[b].T), SX)
            for b in range(BATCH)]
    # interleave hi/lo per row: [D, 2T] with row d = [hi(T) | lo(T)]
    in_maps = []
    for core in range(N_CORES):
        b = core // TP
        tp = core % TP
        heads = np.arange(tp * cfg.h_loc, (tp + 1) * cfg.h_loc)
        ct, st = _rope_tables(cfg.T, sp, heads, 1.0 / (SW * SX))
        cst = np.concatenate([ct, st], axis=1).astype(np.float16)
        rows = (heads[:, None] * HD + perm[None, :]).reshape(-1)
        rows_plain = (heads[:, None] * HD
                      + np.arange(HD)[None, :]).reshape(-1)
        in_maps.append({
            "x8T": x8_b[b],
            "wq8": _hi_lo_pack(np.ascontiguousarray(wq[rows, :].T), SW),
            "wk8": _hi_lo_pack(np.ascontiguousarray(wk[rows, :].T), SW),
            "wv8": _hi_lo_pack(np.ascontiguousarray(wv[rows_plain, :].T),
                               SW),
            "woT": np.ascontiguousarray(wo[:, rows_plain].T
                                        ).astype(np.float16),
            "cst": cst, "tri": tri01,
            "maskexp": me.astype(np.float16),
            "ones": np.ones((128, 1), np.float16),
            "nbias": np.full((128, 1), EXP_BIAS, np.float32),
        })
    return in_maps, kinds


def kernel(x, wq, wk, wv, wo, mask, start_pos):
    cfg = Cfg()
    in_maps, kinds = prepare_in_maps(x, wq, wk, wv, wo, mask, start_pos, cfg)
    nc = get_nc(cfg, kinds)
    out = run_bass_kernel_spmd(nc, in_maps, core_ids=list(range(N_CORES)))
    y = np.zeros((BATCH, SEQ, DIM), np.float32)
    for core in range(N_CORES):
        y[core // TP] += out.results[core]["y"].astype(np.float32)
    return y


# revision 69
# speedup vs baseline: 1.0071x; 1.0071x over previous
"""Trainium2 Bass kernel for nn_Attention_60979945668745.

Multi-head causal attention (B=2, S=2048, D=2048, H=32, hd=64) with
interleaved RoPE, sharded over 8 NeuronCores as DP2 (batch) x TP4 (heads).

Per-core computation (1 batch, 8 heads, feature slice F=512):
  Projections run in fp8e4m3 DoubleRow with hi/lo error compensation:
  operands are split w = wh + wl, x = xh + xl (both parts e4m3); each
  contraction d-pair is covered by 3 DoubleRow matmuls computing
  wh.xh + wl.xh + wh.xl (the lo.lo term is negligible), i.e. 0.75
  PE-cycles per 128-contraction column vs 1.0 for fp16.
  Attention (QK^T, exp, AV) and the output projection run in fp16.

  Per q-chunk c of 512 tokens the kernel emits: projections+RoPE for
  chunk c -> attention rows qc=c (needs only K/V chunks <= c, causal)
  -> output projection for chunk c.  The Tile scheduler overlaps the
  phases (projection matmuls fill PE while attention waits on exp).

RoPE: weight rows are permuted per head to [even dims | odd dims]; the
rotation reads the projection PSUM directly on DVE (a full-tile cos mul,
four 32-partition swapped sin muls -- PSUM sources are exempt from the
same-start-partition BIR rule -- and a Pool add).  The cos/sin tables
fold in the 1/(SW*SX) fp8 dequant scale.  Softmax sums come free via a
ones column appended to V (row 64 of the AV PSUM output); columns are
scaled by 1/sum (reciprocal + partition broadcast) at eviction.  exp
carries a constant bias (cancels after normalization) to keep fp16
ranges safe.
"""

import sys

for _p in ("/opt/trn_rl_repo", "/opt/pypackages"):
    if _p not in sys.path:
        sys.path.insert(0, _p)

import numpy as np
import ml_dtypes

import concourse.bacc as bacc
import concourse.mybir as mybir
from concourse.tile import TileContext
from concourse.bass_utils import run_bass_kernel_spmd

F32 = mybir.dt.float32
F16 = mybir.dt.float16
F8 = mybir.dt.float8e4
AF = mybir.ActivationFunctionType
DR = mybir.MatmulPerfMode.DoubleRow
E4M3 = ml_dtypes.float8_e4m3

DIM = 2048
N_HEADS = 32
HD = 64
BATCH = 2
SEQ = 2048
N_CORES = 8
DP = 2
TP = 4
H_LOC = N_HEADS // TP

EXP_BIAS = -7.0   # cancels in softmax; keeps unnormalized fp16 AV
                  # outputs (evicted before the 1/sum scaling) far from
                  # fp16 overflow
SW = 1024.0   # weight fp8 scale
SX = 16.0     # x fp8 scale


class Cfg:
    def __init__(self, T=SEQ, D=DIM, h_loc=H_LOC, tc=512):
        self.T = T
        self.D = D
        self.h_loc = h_loc
        self.F = h_loc * HD
        self.TC = tc
        self.n_ft = self.F // 128
        self.n_dt = D // 128
        self.n_tc = T // tc
        self.n_kt = T // 128
        self.R = tc // 128

    def key(self):
        return (self.T, self.D, self.h_loc, self.TC)


def build_nc(cfg: Cfg, block_kind, reps=1, dbg=False):
    """block_kind[kt][qc] in {'skip','full','tri','mask'} classifies the
    S^T block (k-tile kt) x (q-chunk qc)."""
    T, D, F, TC = cfg.T, cfg.D, cfg.F, cfg.TC
    n_ft, n_dt, n_tc, n_kt, R = cfg.n_ft, cfg.n_dt, cfg.n_tc, cfg.n_kt, cfg.R
    h_loc = cfg.h_loc

    nc = bacc.Bacc("TRN2", target_bir_lowering=False, debug=False,
                   num_devices=N_CORES)

    # fp8 hi/lo packed operands: rows = d, cols = [hi | lo] interleaved
    x8T = nc.dram_tensor("x8T", [D, 2 * T], F8, kind="ExternalInput")
    wq8 = nc.dram_tensor("wq8", [D, 2 * F], F8, kind="ExternalInput")
    wk8 = nc.dram_tensor("wk8", [D, 2 * F], F8, kind="ExternalInput")
    wv8 = nc.dram_tensor("wv8", [D, 2 * F], F8, kind="ExternalInput")
    woT = nc.dram_tensor("woT", [F, D], F16, kind="ExternalInput")
    cst_d = nc.dram_tensor("cst", [F, 2 * T], F16, kind="ExternalInput")
    tri_d = nc.dram_tensor("tri", [128, 128], F16, kind="ExternalInput")
    ones_d = nc.dram_tensor("ones", [128, 1], F16, kind="ExternalInput")
    nbias_d = nc.dram_tensor("nbias", [128, 1], F32, kind="ExternalInput")
    n_mask = sum(1 for kt in range(n_kt) for qc in range(n_tc)
                 if block_kind[kt][qc] == "mask")
    me_d = nc.dram_tensor("maskexp", [128, max(1, n_mask) * TC], F16,
                          kind="ExternalInput")
    mask_idx = {}
    mi = 0
    for kt in range(n_kt):
        for qc in range(n_tc):
            if block_kind[kt][qc] == "mask":
                mask_idx[(kt, qc)] = mi
                mi += 1

    y = nc.dram_tensor("y", [T, D], F16, kind="ExternalOutput")
    if dbg:
        qh_dump = nc.dram_tensor("qh_dump", [F, T], F16,
                                 kind="ExternalOutput")
        kh_dump = nc.dram_tensor("kh_dump", [F, T], F16,
                                 kind="ExternalOutput")
        v_dump = nc.dram_tensor("v_dump", [128, n_kt * h_loc * 65], F16,
                                kind="ExternalOutput")
        ot_dump = nc.dram_tensor("ot_dump", [F, T], F16,
                                 kind="ExternalOutput")
        rr_dump = nc.dram_tensor("rr_dump", [1, T], F32,
                                 kind="ExternalOutput")

    n_dg = 8                       # d-tiles per DMA/tile group
    n_g = n_dt // n_dg
    n_dc = D // TC

    with TileContext(nc) as tc_:
      with tc_.tile_pool(name="persist", bufs=1) as persist:
        # V_aug: per token-tile [128, h_loc*65]; col 64 of each slot = 1
        v_sb = persist.tile([128, n_kt * h_loc * 65], F16)
        tri_sb = persist.tile([128, 128], F16)
        onesc = persist.tile([128, 1], F16, tag="onesc")
        nbias = persist.tile([128, 1], F32, tag="nbias")
        # resident rope'd Q^T / K^T, feature-major fp16
        qhs = [persist.tile([128, T], F16, tag=f"qh{ft}", name=f"qh{ft}")
               for ft in range(n_ft)]
        khs = [persist.tile([128, T], F16, tag=f"kh{ft}", name=f"kh{ft}")
               for ft in range(n_ft)]
        # per-head attention output (feature-major, fp16, unnormalized
        # then scaled in place)
        ot_sb = persist.tile([128, n_ft * T], F16, tag="ot")
        wo_sb = persist.tile([128, n_ft * D], F16, tag="wo")

        with tc_.tile_pool(name="wr", bufs=1) as wpool, \
             tc_.tile_pool(name="x", bufs=n_g + 1) as xpool, \
             tc_.tile_pool(name="t", bufs=5) as tpool, \
             tc_.tile_pool(name="cs", bufs=n_ft) as cspool, \
             tc_.tile_pool(name="es", bufs=4) as espool, \
             tc_.tile_pool(name="sc", bufs=3) as scpool, \
             tc_.tile_pool(name="ys", bufs=2) as ypool, \
             tc_.tile_pool(name="pj", bufs=2, space="PSUM") as pjpool, \
             tc_.tile_pool(name="pss", bufs=2, space="PSUM") as sspool, \
             tc_.tile_pool(name="po", bufs=2, space="PSUM") as popool:
            # fp8 weights resident: per group g a [128, n_dg*2*F] tile
            wq_ts = [wpool.tile([128, n_dg * 2 * F], F8, tag=f"wq{g}",
                                name=f"wq{g}") for g in range(n_g)]
            wk_ts = [wpool.tile([128, n_dg * 2 * F], F8, tag=f"wk{g}",
                                name=f"wk{g}") for g in range(n_g)]
            wv_ts = [wpool.tile([128, n_dg * 2 * F], F8, tag=f"wv{g}",
                                name=f"wv{g}") for g in range(n_g)]
            def load_w(wts, wdr):
                # group-major so group 0's matmuls can start immediately
                for g in range(n_g):
                    nc.sync.dma_start(
                        out=wts[g][:, :].rearrange(
                            "p (j c) -> p j c", j=n_dg),
                        in_=wdr[g * n_dg * 128:(g + 1) * n_dg * 128,
                                :].rearrange("(j p) c -> p j c", p=128))
            load_w(wq_ts, wq8)
            load_w(wk_ts, wk8)
            nc.sync.dma_start(out=tri_sb[:, :], in_=tri_d[:, :])
            nc.sync.dma_start(out=onesc[:, :], in_=ones_d[:, :])
            nc.sync.dma_start(out=nbias[:, :], in_=nbias_d[:, :])
            ones_view = v_sb[:, :].rearrange("p (n c) -> p n c",
                                             c=65)[:, :, 64]
            nc.vector.tensor_copy(
                ones_view, onesc[:, :].broadcast_to([128, n_kt * h_loc]))

            # ---------------- emission helpers -----------------------
            def emit_attn(qc, h_lo, h_hi):
                kinds = [block_kind[kt][qc] for kt in range(n_kt)]
                live = [kt for kt in range(n_kt) if kinds[kt] != "skip"]
                if not live:
                    return
                rs = rb = None
                for h in range(h_lo, h_hi):
                    po = (h * HD) % 128
                    ft_h = (h * HD) // 128
                    qh = qhs[ft_h][po:po + 64, :]
                    kh = khs[ft_h][po:po + 64, :]
                    osl = ot_sb[po:po + 64, ft_h * T + qc * TC:
                                ft_h * T + (qc + 1) * TC]
                    pso = popool.tile([128, TC], F32, tag="pso")
                    first = True
                    pairs = [live[i:i + 2] for i in range(0, len(live), 2)]
                    for pair in pairs:
                        ps = sspool.tile([128, 2 * TC], F32, tag="pss")
                        es = espool.tile([128, 2 * TC], F16, tag="es")
                        # valid block ih occupies ps cols
                        # [base(ih) : base(ih) + TC - off(ih)] (packed
                        # contiguously so exp covers no causal garbage)
                        offs, bases = [], []
                        pos = 0
                        for kt in pair:
                            off = (max(0, kt * 128 - qc * TC)
                                   if kinds[kt] == "tri" else 0)
                            offs.append(off)
                            bases.append(pos)
                            pos += TC - off
                        for ih, kt in enumerate(pair):
                            off, base = offs[ih], bases[ih]
                            nc.tensor.matmul(
                                ps[:, base:base + TC - off],
                                kh[:, kt * 128:(kt + 1) * 128],
                                qh[:, qc * TC + off:(qc + 1) * TC],
                                start=True, stop=True)
                        nc.scalar.activation(es[:, 0:pos], ps[:, 0:pos],
                                             AF.Exp, scale=0.125,
                                             bias=nbias[:, :])
                        for ih, kt in enumerate(pair):
                            off, base = offs[ih], bases[ih]
                            if kinds[kt] == "tri":
                                nc.vector.tensor_mul(
                                    es[:, base:base + 128],
                                    es[:, base:base + 128],
                                    tri_sb[:, :])
                            elif kinds[kt] == "mask":
                                mslab = mask_idx[(kt, qc)]
                                mt = espool.tile([128, TC], F16, tag="mt")
                                nc.sync.dma_start(
                                    out=mt[:, :],
                                    in_=me_d[:, mslab * TC:
                                             (mslab + 1) * TC])
                                nc.vector.tensor_mul(
                                    es[:, base:base + TC],
                                    es[:, base:base + TC],
                                    mt[:, :])
                            nc.tensor.matmul(
                                pso[0:65, off:TC],
                                v_sb[:, kt * h_loc * 65 + h * 65:
                                     kt * h_loc * 65 + h * 65 + 65],
                                es[:, base:base + TC - off],
                                start=first,
                                stop=(kt == live[-1]))
                            first = False
                    # normalize columns by 1/sums (row 64) at eviction
                    rs = scpool.tile([1, TC], F32, tag="rs")
                    nc.vector.tensor_copy(rs[:, :], pso[64:65, :])
                    rr = scpool.tile([1, TC], F32, tag="rr")
                    nc.vector.reciprocal_approx_fast(rr[:, :], rs[:, :])
                    rb = scpool.tile([64, TC], F32, tag="rb")
                    nc.gpsimd.partition_broadcast(rb[:, :], rr[:, :])
                    if dbg and h == 0 and qc == 0:
                        nc.sync.dma_start(
                            out=rr_dump[:, qc * TC:(qc + 1) * TC],
                            in_=rr[:, :])
                    nc.vector.tensor_mul(osl, pso[0:64, :], rb[:, :])

            def emit_p3(qc, evict_eng=None):
                if evict_eng is None:
                    def evict(out, in_):
                        nc.scalar.activation(out, in_, AF.Copy)
                else:
                    evict = evict_eng.tensor_copy
                for ttile in range(qc * R, (qc + 1) * R):
                    ysrow = ypool.tile([128, D], F16, tag="ysrow")
                    for dc in range(n_dc):
                        psy = pjpool.tile([128, TC], F32, tag="ps")
                        for fk in range(n_ft):
                            nc.tensor.matmul(
                                psy[:, :],
                                ot_sb[:, fk * T + ttile * 128:
                                      fk * T + (ttile + 1) * 128],
                                wo_sb[:, fk * D + dc * TC:
                                      fk * D + (dc + 1) * TC],
                                start=(fk == 0),
                                stop=(fk == n_ft - 1))
                        evict(ysrow[:, dc * TC:(dc + 1) * TC], psy[:, :])
                    nc.sync.dma_start(
                        out=y[ttile * 128:(ttile + 1) * 128, :],
                        in_=ysrow[:, :])

            for _rep in range(reps):
              for c in range(n_tc):
                cs = slice(c * TC, (c + 1) * TC)
                # ============ phase 1: projections + RoPE, chunk c =======
                xg = []
                for g in range(n_g):
                    xtile = xpool.tile([128, n_dg * 2 * TC], F8, tag="xt")
                    xin = x8T[g * n_dg * 128:(g + 1) * n_dg * 128,
                              :].rearrange("(j p) (two t) -> p j two t",
                                           p=128, two=2)
                    xout = xtile[:, :].rearrange(
                        "p (j two t) -> p j two t", j=n_dg, two=2)
                    for hl in range(2):
                        nc.scalar.dma_start(out=xout[:, :, hl, :],
                                            in_=xin[:, :, hl, cs])
                    xg.append(xtile)
                if c == 0:
                    load_w(wv_ts, wv8)
                    for fk in range(n_ft):
                        nc.sync.dma_start(
                            out=wo_sb[:, fk * D:(fk + 1) * D],
                            in_=woT[fk * 128:(fk + 1) * 128, :])

                def wv4(wts, g):
                    return wts[g][:, :].rearrange(
                        "p (j two f) -> p j two f", j=n_dg, two=2)

                def xv4(g):
                    return xg[g][:, :].rearrange(
                        "p (j two t) -> p j two t", j=n_dg, two=2)

                # cos/sin slabs for this chunk, shared by Q and K
                cst_c = []
                for ft in range(n_ft):
                    cst_sb = cspool.tile([128, 2 * TC], F16, tag="cst")
                    nc.scalar.dma_start(
                        out=cst_sb[:, :].rearrange(
                            "p (two t) -> p two t", two=2),
                        in_=cst_d[ft * 128:(ft + 1) * 128, :].rearrange(
                            "p (two t) -> p two t", two=2)[:, :, cs])
                    cst_c.append(cst_sb)

                def emit_qk_group(wts, dest, ft, pool=None, ptag="ps"):
                    # Q/K feature-major with fused RoPE
                    fs = slice(ft * 128, (ft + 1) * 128)
                    ps = (pool or pjpool).tile([128, TC], F32, tag=ptag)
                    for dp in range(0, n_dt, 2):
                        g, j = divmod(dp, n_dg)
                        w4 = wv4(wts, g)
                        x4 = xv4(g)
                        wh = w4[:, j:j + 2, 0, fs]
                        wl = w4[:, j:j + 2, 1, fs]
                        xh = x4[:, j:j + 2, 0, :]
                        xl = x4[:, j:j + 2, 1, :]
                        nc.tensor.matmul(ps[:, :], wh, xh, perf_mode=DR,
                                         start=(dp == 0), stop=False)
                        nc.tensor.matmul(ps[:, :], wl, xh, perf_mode=DR,
                                         start=False, stop=False)
                        nc.tensor.matmul(ps[:, :], wh, xl, perf_mode=DR,
                                         start=False,
                                         stop=(dp == n_dt - 2))
                    # RoPE: dest[a] = ps[a]*ct[a] + ps[a^32]*st[a]
                    # (swapped operand reads PSUM -- partition-shifted
                    # SBUF operands fail BIR verification)
                    cst_sb = cst_c[ft]
                    t1 = tpool.tile([128, TC], F16, tag="t1")
                    nc.vector.tensor_mul(t1[:, :], ps[:, :],
                                         cst_sb[:, 0:TC])
                    qs = tpool.tile([128, TC], F16, tag="qs")
                    for s in range(4):
                        a, b = s * 32, (s ^ 1) * 32
                        nc.vector.tensor_mul(
                            qs[a:a + 32, :], ps[b:b + 32, :],
                            cst_sb[a:a + 32, TC:2 * TC])
                    nc.gpsimd.tensor_add(
                        dest[ft][:, cs], t1[:, :], qs[:, :])

                def emit_v_group(tt):
                    # V token-major (stationary = x token slices)
                    kt = c * R + tt
                    psv = pjpool.tile([128, F], F32, tag="ps")
                    for dp in range(0, n_dt, 2):
                        g, j = divmod(dp, n_dg)
                        w4 = wv4(wv_ts, g)
                        x4 = xv4(g)
                        tsl = slice(tt * 128, (tt + 1) * 128)
                        xh = x4[:, j:j + 2, 0, tsl]
                        xl = x4[:, j:j + 2, 1, tsl]
                        wh = w4[:, j:j + 2, 0, :]
                        wl = w4[:, j:j + 2, 1, :]
                        nc.tensor.matmul(psv[:, :], xh, wh, perf_mode=DR,
                                         start=(dp == 0), stop=False)
                        nc.tensor.matmul(psv[:, :], xl, wh, perf_mode=DR,
                                         start=False, stop=False)
                        nc.tensor.matmul(psv[:, :], xh, wl, perf_mode=DR,
                                         start=False,
                                         stop=(dp == n_dt - 2))
                    base = kt * h_loc * 65
                    vout = v_sb[:, base:base + h_loc * 65]
                    vout = vout.rearrange(
                        "p (h c) -> p h c", c=65)[:, :, 0:64]
                    vin = psv[:, :].rearrange("p (h c) -> p h c", c=64)
                    nc.scalar.activation(vout, vin, AF.Copy,
                                         scale=1.0 / (SW * SX))

                for ft in range(n_ft):
                    emit_qk_group(wq_ts, qhs, ft)
                for ft in range(n_ft):
                    emit_qk_group(wk_ts, khs, ft)
                for tt in range(R):
                    emit_v_group(tt)

                # ---- attention staggered at half-row granularity: each
                # row's first half right after its chunk, second half one
                # chunk later; keeps exp (ACT) uniformly loaded while the
                # next chunk's projections fill the PE.  The output
                # projection (pure PE, no ACT) is all deferred to the end
                # where it fills the exp-wait gaps of the late rows.
                hh = h_loc // 2
                if c >= 1:
                    emit_attn(c - 1, hh, h_loc)
                emit_attn(c, 0, hh)
              emit_attn(n_tc - 1, hh, h_loc)
              for qc in range(n_tc):
                  emit_p3(qc, evict_eng=nc.vector if qc == n_tc - 1
                          else None)
              if dbg:
                nc.sync.dma_start(out=v_dump[:, :], in_=v_sb[:, :])
                for ft in range(n_ft):
                    nc.sync.dma_start(
                        out=qh_dump[ft * 128:(ft + 1) * 128, :],
                        in_=qhs[ft][:, :])
                    nc.sync.dma_start(
                        out=kh_dump[ft * 128:(ft + 1) * 128, :],
                        in_=khs[ft][:, :])
                    nc.sync.dma_start(
                        out=ot_dump[ft * 128:(ft + 1) * 128, :],
                        in_=ot_sb[:, ft * T:(ft + 1) * T])

    nc.compile()
    return nc


# ---------------- host-side preparation ----------------

def _rope_tables(T, start_pos, heads, scale):
    """Reference RoPE uses a PER-HEAD angle: theta = base[head] * pos.
    Returns [len(heads)*64, T] fp32 ct / st tables in the permuted
    [evens|odds] feature order, times `scale` (fp8 dequant fold).
    st is TARGET-indexed: even-half rows carry -s, odd-half +s
    (qs[a] = ps[a^32] * st[a])."""
    base = 1.0 / (10000.0 ** (np.arange(0, HD, 2, dtype=np.float64) / HD))
    pos = start_pos + np.arange(T, dtype=np.float64)
    F_ = len(heads) * HD
    ct = np.empty((F_, T), np.float32)
    st = np.empty((F_, T), np.float32)
    for i, g in enumerate(heads):
        th = base[g] * pos                        # [T]
        c, s = np.cos(th) * scale, np.sin(th) * scale
        b = i * HD
        ct[b:b + 64] = c[None, :]
        st[b:b + 32] = -s[None, :]
        st[b + 32:b + 64] = s[None, :]
    return ct, st


def _perm():
    p = np.empty(HD, np.int64)
    p[:32] = np.arange(0, HD, 2)
    p[32:] = np.arange(1, HD, 2)
    return p


def _hi_lo_pack(a, scale):
    """a: [rows, cols] fp32 -> [rows, 2*cols] e4m3 (hi | lo per row)."""
    s = (a * scale).astype(np.float32)
    hi = s.astype(E4M3)
    lo = (s - hi.astype(np.float32)).astype(E4M3)
    out = np.empty((a.shape[0], 2 * a.shape[1]), E4M3)
    out[:, :a.shape[1]] = hi
    out[:, a.shape[1]:] = lo
    return out


def _classify_mask(mask, T, TC):
    """mask: [S,S] additive (rows=q, cols=k). Returns block_kind[kt][qc] and
    packed exp-mask slabs ([k,q] orientation) for 'mask' blocks."""
    n_kt = T // 128
    n_tc = T // TC
    tri = (np.arange(TC)[None, :] >= np.arange(128)[:, None])
    kinds = [[None] * n_tc for _ in range(n_kt)]
    for kt in range(n_kt):
        for qc in range(n_tc):
            blk = mask[qc * TC:(qc + 1) * TC, kt * 128:(kt + 1) * 128].T
            if np.all(blk <= -1e8):
                kinds[kt][qc] = "skip"
            elif np.all(blk == 0.0):
                kinds[kt][qc] = "full"
            else:
                off = kt * 128 - qc * TC
                is_tri = False
                if 0 <= off <= TC - 128:
                    ref = np.full((128, TC), -1e9, np.float32)
                    ref[:, off:] = np.where(tri[:, :TC - off], 0.0, -1e9)
                    is_tri = bool(np.array_equal(blk, ref))
                kinds[kt][qc] = "tri" if is_tri else "mask"
    # a 'tri' block may not open an accumulation group at off>0
    for qc in range(n_tc):
        for kt in range(n_kt):
            k = kinds[kt][qc]
            if k == "skip":
                continue
            if k == "tri" and kt * 128 - qc * TC > 0:
                kinds[kt][qc] = "mask"
            break
    slabs = []
    for kt in range(n_kt):
        for qc in range(n_tc):
            if kinds[kt][qc] == "mask":
                blk = mask[qc * TC:(qc + 1) * TC,
                           kt * 128:(kt + 1) * 128].T
                slabs.append(np.exp(blk.astype(np.float64)
                                    ).astype(np.float32))
    me = (np.concatenate(slabs, axis=1) if slabs
          else np.zeros((128, TC), np.float32))
    return kinds, me


_CACHE = {}


def get_nc(cfg: Cfg, block_kind):
    key = (cfg.key(), tuple(tuple(r) for r in block_kind))
    if key not in _CACHE:
        _CACHE[key] = build_nc(cfg, block_kind)
    return _CACHE[key]


def prepare_in_maps(x, wq, wk, wv, wo, mask, start_pos, cfg):
    x = np.asarray(x, np.float32)
    wq = np.asarray(wq, np.float32)
    wk = np.asarray(wk, np.float32)
    wv = np.asarray(wv, np.float32)
    wo = np.asarray(wo, np.float32)
    mask2d = np.asarray(mask, np.float32).reshape(mask.shape[-2],
                                                  mask.shape[-1])
    sp = int(np.asarray(start_pos))

    tri01 = (np.arange(128)[None, :] >= np.arange(128)[:, None]
             ).astype(np.float16)
    kinds, me = _classify_mask(mask2d, cfg.T, cfg.TC)

    perm = _perm()
    x8_b = [_hi_lo_pack(np.ascontiguousarray(x[b].T), SX)
            for b in range(BATCH)]
    # interleave hi/lo per row: [D, 2T] with row d = [hi(T) | lo(T)]
    in_maps = []
    for core in range(N_CORES):
        b = core // TP
        tp = core % TP
        heads = np.arange(tp * cfg.h_loc, (tp + 1) * cfg.h_loc)
        ct, st = _rope_tables(cfg.T, sp, heads, 1.0 / (SW * SX))
        cst = np.concatenate([ct, st], axis=1).astype(np.float16)
        rows = (heads[:, None] * HD + perm[None, :]).reshape(-1)
        rows_plain = (heads[:, None] * HD
                      + np.arange(HD)[None, :]).reshape(-1)
        in_maps.append({
            "x8T": x8_b[b],
            "wq8": _hi_lo_pack(np.ascontiguousarray(wq[rows, :].T), SW),
            "wk8": _hi_lo_pack(np.ascontiguousarray(wk[rows, :].T), SW),
            "wv8": _hi_lo_pack(np.ascontiguousarray(wv[rows_plain, :].T),
                               SW),
            "woT": np.ascontiguousarray(wo[:, rows_plain].T
                                        ).astype(np.float16),
            "cst": cst, "tri": tri01,
            "maskexp": me.astype(np.float16),
            "ones": np.ones((128, 1), np.float16),
            "nbias": np.full((128, 1), EXP_BIAS, np.float32),
        })
    return in_maps, kinds


def kernel(x, wq, wk, wv, wo, mask, start_pos):
    cfg = Cfg()
    in_maps, kinds = prepare_in_maps(x, wq, wk, wv, wo, mask, start_pos, cfg)
    nc = get_nc(cfg, kinds)
    out = run_bass_kernel_spmd(nc, in_maps, core_ids=list(range(N_CORES)))
    y = np.zeros((BATCH, SEQ, DIM), np.float32)
    for core in range(N_CORES):
        y[core // TP] += out.results[core]["y"].astype(np.float32)
    return y


# revision 93
# speedup vs baseline: 1.0282x; 1.0210x over previous
"""Trainium2 Bass kernel for nn_Attention_60979945668745.

Multi-head causal attention (B=2, S=2048, D=2048, H=32, hd=64) with
interleaved RoPE, sharded over 8 NeuronCores as DP2 (batch) x TP4 (heads).

Per-core computation (1 batch, 8 heads, feature slice F=512):
  Projections run in fp8e4m3 DoubleRow with hi/lo error compensation:
  operands are split w = wh + wl, x = xh + xl (both parts e4m3); each
  contraction d-pair is covered by 3 DoubleRow matmuls computing
  wh.xh + wl.xh + wh.xl (the lo.lo term is negligible), i.e. 0.75
  PE-cycles per 128-contraction column vs 1.0 for fp16.
  Attention (QK^T, exp, AV) and the output projection run in fp16.

  Per q-chunk c of 512 tokens the kernel emits: projections+RoPE for
  chunk c -> attention rows qc=c (needs only K/V chunks <= c, causal)
  -> output projection for chunk c.  The Tile scheduler overlaps the
  phases (projection matmuls fill PE while attention waits on exp).

RoPE: weight rows are permuted per head to [even dims | odd dims].  The
cos product runs on an fp16 ACT eviction of the projection PSUM (the
1/(SW*SX) fp8 dequant folds into the copy scale -- the raw PSUM is in
the scaled domain and would overflow fp16) so it hits the DVE 2-byte
fast path; the four 32-partition swapped sin muls read the PSUM
directly (PSUM sources are exempt from the same-start-partition BIR
rule) with the dequant folded into the sin table; a Pool add combines
them.  Softmax sums come free via a ones column appended to V (row 64
of the AV PSUM output); columns are scaled by 1/sum (reciprocal +
partition broadcast) at eviction.  exp carries a constant bias (cancels
after normalization) to keep fp16 ranges safe.
"""

import sys

for _p in ("/opt/trn_rl_repo", "/opt/pypackages"):
    if _p not in sys.path:
        sys.path.insert(0, _p)

import numpy as np
import ml_dtypes

import concourse.bacc as bacc
import concourse.mybir as mybir
from concourse.tile import TileContext
from concourse.bass_utils import run_bass_kernel_spmd

F32 = mybir.dt.float32
F16 = mybir.dt.float16
F8 = mybir.dt.float8e4
AF = mybir.ActivationFunctionType
DR = mybir.MatmulPerfMode.DoubleRow
E4M3 = ml_dtypes.float8_e4m3

DIM = 2048
N_HEADS = 32
HD = 64
BATCH = 2
SEQ = 2048
N_CORES = 8
DP = 2
TP = 4
H_LOC = N_HEADS // TP

EXP_BIAS = -7.0   # cancels in softmax; keeps unnormalized fp16 AV
                  # outputs (evicted before the 1/sum scaling) far from
                  # fp16 overflow
SW = 1024.0   # weight fp8 scale
SX = 16.0     # x fp8 scale


class Cfg:
    def __init__(self, T=SEQ, D=DIM, h_loc=H_LOC, tc=512):
        self.T = T
        self.D = D
        self.h_loc = h_loc
        self.F = h_loc * HD
        self.TC = tc
        self.n_ft = self.F // 128
        self.n_dt = D // 128
        self.n_tc = T // tc
        self.n_kt = T // 128
        self.R = tc // 128

    def key(self):
        return (self.T, self.D, self.h_loc, self.TC)


def build_nc(cfg: Cfg, block_kind, reps=1, dbg=False):
    """block_kind[kt][qc] in {'skip','full','tri','mask'} classifies the
    S^T block (k-tile kt) x (q-chunk qc)."""
    T, D, F, TC = cfg.T, cfg.D, cfg.F, cfg.TC
    n_ft, n_dt, n_tc, n_kt, R = cfg.n_ft, cfg.n_dt, cfg.n_tc, cfg.n_kt, cfg.R
    h_loc = cfg.h_loc

    nc = bacc.Bacc("TRN2", target_bir_lowering=False, debug=False,
                   num_devices=N_CORES)

    # fp8 hi/lo packed operands: rows = d, cols = [hi | lo] interleaved
    x8T = nc.dram_tensor("x8T", [D, 2 * T], F8, kind="ExternalInput")
    wq8 = nc.dram_tensor("wq8", [D, 2 * F], F8, kind="ExternalInput")
    wk8 = nc.dram_tensor("wk8", [D, 2 * F], F8, kind="ExternalInput")
    wv8 = nc.dram_tensor("wv8", [D, 2 * F], F8, kind="ExternalInput")
    woT = nc.dram_tensor("woT", [F, D], F16, kind="ExternalInput")
    cst_d = nc.dram_tensor("cst", [F, 2 * T], F16, kind="ExternalInput")
    tri_d = nc.dram_tensor("tri", [128, 128], F16, kind="ExternalInput")
    ones_d = nc.dram_tensor("ones", [128, 1], F16, kind="ExternalInput")
    nbias_d = nc.dram_tensor("nbias", [128, 1], F32, kind="ExternalInput")
    n_mask = sum(1 for kt in range(n_kt) for qc in range(n_tc)
                 if block_kind[kt][qc] == "mask")
    me_d = nc.dram_tensor("maskexp", [128, max(1, n_mask) * TC], F16,
                          kind="ExternalInput")
    mask_idx = {}
    mi = 0
    for kt in range(n_kt):
        for qc in range(n_tc):
            if block_kind[kt][qc] == "mask":
                mask_idx[(kt, qc)] = mi
                mi += 1

    y = nc.dram_tensor("y", [T, D], F16, kind="ExternalOutput")
    if dbg:
        qh_dump = nc.dram_tensor("qh_dump", [F, T], F16,
                                 kind="ExternalOutput")
        kh_dump = nc.dram_tensor("kh_dump", [F, T], F16,
                                 kind="ExternalOutput")
        v_dump = nc.dram_tensor("v_dump", [128, n_kt * h_loc * 65], F16,
                                kind="ExternalOutput")
        ot_dump = nc.dram_tensor("ot_dump", [F, T], F16,
                                 kind="ExternalOutput")
        rr_dump = nc.dram_tensor("rr_dump", [1, T], F32,
                                 kind="ExternalOutput")

    n_dg = 8                       # d-tiles per DMA/tile group
    n_g = n_dt // n_dg
    n_dc = D // TC

    with TileContext(nc) as tc_:
      with tc_.tile_pool(name="persist", bufs=1) as persist:
        # V_aug: per token-tile [128, h_loc*65]; col 64 of each slot = 1
        v_sb = persist.tile([128, n_kt * h_loc * 65], F16)
        tri_sb = persist.tile([128, 128], F16)
        onesc = persist.tile([128, 1], F16, tag="onesc")
        nbias = persist.tile([128, 1], F32, tag="nbias")
        # resident rope'd Q^T / K^T, feature-major fp16
        qhs = [persist.tile([128, T], F16, tag=f"qh{ft}", name=f"qh{ft}")
               for ft in range(n_ft)]
        khs = [persist.tile([128, T], F16, tag=f"kh{ft}", name=f"kh{ft}")
               for ft in range(n_ft)]
        # per-head attention output (feature-major, fp16, unnormalized
        # then scaled in place)
        ot_sb = persist.tile([128, n_ft * T], F16, tag="ot")
        wo_sb = persist.tile([128, n_ft * D], F16, tag="wo")

        with tc_.tile_pool(name="wr", bufs=1) as wpool, \
             tc_.tile_pool(name="x", bufs=n_g + 1) as xpool, \
             tc_.tile_pool(name="t", bufs=4) as tpool, \
             tc_.tile_pool(name="cs", bufs=n_ft) as cspool, \
             tc_.tile_pool(name="es", bufs=4) as espool, \
             tc_.tile_pool(name="sc", bufs=3) as scpool, \
             tc_.tile_pool(name="ys", bufs=2) as ypool, \
             tc_.tile_pool(name="pj", bufs=2, space="PSUM") as pjpool, \
             tc_.tile_pool(name="pss", bufs=2, space="PSUM") as sspool, \
             tc_.tile_pool(name="po", bufs=2, space="PSUM") as popool:
            # fp8 weights resident: per group g a [128, n_dg*2*F] tile
            wq_ts = [wpool.tile([128, n_dg * 2 * F], F8, tag=f"wq{g}",
                                name=f"wq{g}") for g in range(n_g)]
            wk_ts = [wpool.tile([128, n_dg * 2 * F], F8, tag=f"wk{g}",
                                name=f"wk{g}") for g in range(n_g)]
            wv_ts = [wpool.tile([128, n_dg * 2 * F], F8, tag=f"wv{g}",
                                name=f"wv{g}") for g in range(n_g)]
            def load_w(wts, wdr):
                # group-major so group 0's matmuls can start immediately
                for g in range(n_g):
                    nc.sync.dma_start(
                        out=wts[g][:, :].rearrange(
                            "p (j c) -> p j c", j=n_dg),
                        in_=wdr[g * n_dg * 128:(g + 1) * n_dg * 128,
                                :].rearrange("(j p) c -> p j c", p=128))
            load_w(wq_ts, wq8)
            load_w(wk_ts, wk8)
            nc.sync.dma_start(out=tri_sb[:, :], in_=tri_d[:, :])
            nc.sync.dma_start(out=onesc[:, :], in_=ones_d[:, :])
            nc.sync.dma_start(out=nbias[:, :], in_=nbias_d[:, :])
            ones_view = v_sb[:, :].rearrange("p (n c) -> p n c",
                                             c=65)[:, :, 64]
            nc.vector.tensor_copy(
                ones_view, onesc[:, :].broadcast_to([128, n_kt * h_loc]))

            # ---------------- emission helpers -----------------------
            def emit_attn(qc, h_lo, h_hi):
                kinds = [block_kind[kt][qc] for kt in range(n_kt)]
                live = [kt for kt in range(n_kt) if kinds[kt] != "skip"]
                if not live:
                    return
                rs = rb = None
                for h in range(h_lo, h_hi):
                    po = (h * HD) % 128
                    ft_h = (h * HD) // 128
                    qh = qhs[ft_h][po:po + 64, :]
                    kh = khs[ft_h][po:po + 64, :]
                    osl = ot_sb[po:po + 64, ft_h * T + qc * TC:
                                ft_h * T + (qc + 1) * TC]
                    pso = popool.tile([128, TC], F32, tag="pso")
                    first = True
                    pairs = [live[i:i + 2] for i in range(0, len(live), 2)]
                    for pair in pairs:
                        ps = sspool.tile([128, 2 * TC], F32, tag="pss")
                        es = espool.tile([128, 2 * TC], F16, tag="es")
                        # valid block ih occupies ps cols
                        # [base(ih) : base(ih) + TC - off(ih)] (packed
                        # contiguously so exp covers no causal garbage)
                        offs, bases = [], []
                        pos = 0
                        for kt in pair:
                            off = (max(0, kt * 128 - qc * TC)
                                   if kinds[kt] == "tri" else 0)
                            offs.append(off)
                            bases.append(pos)
                            pos += TC - off
                        for ih, kt in enumerate(pair):
                            off, base = offs[ih], bases[ih]
                            nc.tensor.matmul(
                                ps[:, base:base + TC - off],
                                kh[:, kt * 128:(kt + 1) * 128],
                                qh[:, qc * TC + off:(qc + 1) * TC],
                                start=True, stop=True)
                        nc.scalar.activation(es[:, 0:pos], ps[:, 0:pos],
                                             AF.Exp, scale=0.125,
                                             bias=nbias[:, :])
                        for ih, kt in enumerate(pair):
                            off, base = offs[ih], bases[ih]
                            if kinds[kt] == "tri":
                                nc.vector.tensor_mul(
                                    es[:, base:base + 128],
                                    es[:, base:base + 128],
                                    tri_sb[:, :])
                            elif kinds[kt] == "mask":
                                mslab = mask_idx[(kt, qc)]
                                mt = espool.tile([128, TC], F16, tag="mt")
                                nc.sync.dma_start(
                                    out=mt[:, :],
                                    in_=me_d[:, mslab * TC:
                                             (mslab + 1) * TC])
                                nc.vector.tensor_mul(
                                    es[:, base:base + TC],
                                    es[:, base:base + TC],
                                    mt[:, :])
                            nc.tensor.matmul(
                                pso[0:65, off:TC],
                                v_sb[:, kt * h_loc * 65 + h * 65:
                                     kt * h_loc * 65 + h * 65 + 65],
                                es[:, base:base + TC - off],
                                start=first,
                                stop=(kt == live[-1]))
                            first = False
                    # normalize columns by 1/sums (row 64) at eviction
                    rs = scpool.tile([1, TC], F32, tag="rs")
                    nc.vector.tensor_copy(rs[:, :], pso[64:65, :])
                    rr = scpool.tile([1, TC], F32, tag="rr")
                    nc.vector.reciprocal_approx_fast(rr[:, :], rs[:, :])
                    rb = scpool.tile([64, TC], F32, tag="rb")
                    nc.gpsimd.partition_broadcast(rb[:, :], rr[:, :])
                    if dbg and h == 0 and qc == 0:
                        nc.sync.dma_start(
                            out=rr_dump[:, qc * TC:(qc + 1) * TC],
                            in_=rr[:, :])
                    nc.vector.tensor_mul(osl, pso[0:64, :], rb[:, :])

            def emit_p3(qc, evict_eng=None):
                if evict_eng is None:
                    def evict(out, in_):
                        nc.scalar.activation(out, in_, AF.Copy)
                else:
                    evict = evict_eng.tensor_copy
                for ttile in range(qc * R, (qc + 1) * R):
                    ysrow = ypool.tile([128, D], F16, tag="ysrow")
                    for dc in range(n_dc):
                        psy = pjpool.tile([128, TC], F32, tag="ps")
                        for fk in range(n_ft):
                            nc.tensor.matmul(
                                psy[:, :],
                                ot_sb[:, fk * T + ttile * 128:
                                      fk * T + (ttile + 1) * 128],
                                wo_sb[:, fk * D + dc * TC:
                                      fk * D + (dc + 1) * TC],
                                start=(fk == 0),
                                stop=(fk == n_ft - 1))
                        evict(ysrow[:, dc * TC:(dc + 1) * TC], psy[:, :])
                    nc.sync.dma_start(
                        out=y[ttile * 128:(ttile + 1) * 128, :],
                        in_=ysrow[:, :])

            for _rep in range(reps):
              for c in range(n_tc):
                cs = slice(c * TC, (c + 1) * TC)
                # ============ phase 1: projections + RoPE, chunk c =======
                xg = []
                for g in range(n_g):
                    xtile = xpool.tile([128, n_dg * 2 * TC], F8, tag="xt")
                    xin = x8T[g * n_dg * 128:(g + 1) * n_dg * 128,
                              :].rearrange("(j p) (two t) -> p j two t",
                                           p=128, two=2)
                    xout = xtile[:, :].rearrange(
                        "p (j two t) -> p j two t", j=n_dg, two=2)
                    for hl in range(2):
                        nc.scalar.dma_start(out=xout[:, :, hl, :],
                                            in_=xin[:, :, hl, cs])
                    xg.append(xtile)
                if c == 0:
                    load_w(wv_ts, wv8)
                    for fk in range(n_ft):
                        nc.sync.dma_start(
                            out=wo_sb[:, fk * D:(fk + 1) * D],
                            in_=woT[fk * 128:(fk + 1) * 128, :])

                def wv4(wts, g):
                    return wts[g][:, :].rearrange(
                        "p (j two f) -> p j two f", j=n_dg, two=2)

                def xv4(g):
                    return xg[g][:, :].rearrange(
                        "p (j two t) -> p j two t", j=n_dg, two=2)

                # cos/sin slabs for this chunk, shared by Q and K
                cst_c = []
                for ft in range(n_ft):
                    cst_sb = cspool.tile([128, 2 * TC], F16, tag="cst")
                    nc.scalar.dma_start(
                        out=cst_sb[:, :].rearrange(
                            "p (two t) -> p two t", two=2),
                        in_=cst_d[ft * 128:(ft + 1) * 128, :].rearrange(
                            "p (two t) -> p two t", two=2)[:, :, cs])
                    cst_c.append(cst_sb)

                def emit_qk_group(wts, dest, ft, pool=None, ptag="ps"):
                    # Q/K feature-major with fused RoPE
                    fs = slice(ft * 128, (ft + 1) * 128)
                    ps = (pool or pjpool).tile([128, TC], F32, tag=ptag)
                    for dp in range(0, n_dt, 2):
                        g, j = divmod(dp, n_dg)
                        w4 = wv4(wts, g)
                        x4 = xv4(g)
                        wh = w4[:, j:j + 2, 0, fs]
                        wl = w4[:, j:j + 2, 1, fs]
                        xh = x4[:, j:j + 2, 0, :]
                        xl = x4[:, j:j + 2, 1, :]
                        nc.tensor.matmul(ps[:, :], wh, xh, perf_mode=DR,
                                         start=(dp == 0), stop=False)
                        nc.tensor.matmul(ps[:, :], wl, xh, perf_mode=DR,
                                         start=False, stop=False)
                        nc.tensor.matmul(ps[:, :], wh, xl, perf_mode=DR,
                                         start=False,
                                         stop=(dp == n_dt - 2))
                    # RoPE: dest[a] = ps[a]*ct[a] + ps[a^32]*st[a]
                    # (swapped operand reads PSUM -- partition-shifted
                    # SBUF operands fail BIR verification).  The aligned
                    # cos product runs on an fp16 copy (ACT, idle in P1)
                    # so it hits the DVE 2-byte fast path.
                    cst_sb = cst_c[ft]
                    se = tpool.tile([128, TC], F16, tag="se")
                    # dequant here: ps is in the scaled fp8 domain and
                    # would overflow fp16 without the 1/(SW*SX) fold
                    nc.scalar.activation(se[:, :], ps[:, :], AF.Copy,
                                         scale=1.0 / (SW * SX))
                    t1 = tpool.tile([128, TC], F16, tag="t1")
                    nc.vector.tensor_mul(t1[:, :], se[:, :],
                                         cst_sb[:, 0:TC])
                    qs = tpool.tile([128, TC], F16, tag="qs")
                    for s in range(4):
                        a, b = s * 32, (s ^ 1) * 32
                        nc.vector.tensor_mul(
                            qs[a:a + 32, :], ps[b:b + 32, :],
                            cst_sb[a:a + 32, TC:2 * TC])
                    nc.gpsimd.tensor_add(
                        dest[ft][:, cs], t1[:, :], qs[:, :])

                def emit_v_group(tt):
                    # V token-major (stationary = x token slices)
                    kt = c * R + tt
                    psv = pjpool.tile([128, F], F32, tag="ps")
                    for dp in range(0, n_dt, 2):
                        g, j = divmod(dp, n_dg)
                        w4 = wv4(wv_ts, g)
                        x4 = xv4(g)
                        tsl = slice(tt * 128, (tt + 1) * 128)
                        xh = x4[:, j:j + 2, 0, tsl]
                        xl = x4[:, j:j + 2, 1, tsl]
                        wh = w4[:, j:j + 2, 0, :]
                        wl = w4[:, j:j + 2, 1, :]
                        nc.tensor.matmul(psv[:, :], xh, wh, perf_mode=DR,
                                         start=(dp == 0), stop=False)
                        nc.tensor.matmul(psv[:, :], xl, wh, perf_mode=DR,
                                         start=False, stop=False)
                        nc.tensor.matmul(psv[:, :], xh, wl, perf_mode=DR,
                                         start=False,
                                         stop=(dp == n_dt - 2))
                    base = kt * h_loc * 65
                    vout = v_sb[:, base:base + h_loc * 65]
                    vout = vout.rearrange(
                        "p (h c) -> p h c", c=65)[:, :, 0:64]
                    vin = psv[:, :].rearrange("p (h c) -> p h c", c=64)
                    nc.scalar.activation(vout, vin, AF.Copy,
                                         scale=1.0 / (SW * SX))

                for ft in range(n_ft):
                    emit_qk_group(wq_ts, qhs, ft)
                for ft in range(n_ft):
                    emit_qk_group(wk_ts, khs, ft)
                for tt in range(R):
                    emit_v_group(tt)

                # ---- attention staggered at half-row granularity: each
                # row's first half right after its chunk, second half one
                # chunk later; keeps exp (ACT) uniformly loaded while the
                # next chunk's projections fill the PE.  The output
                # projection (pure PE, no ACT) is all deferred to the end
                # where it fills the exp-wait gaps of the late rows.
                hh = h_loc // 2
                if c >= 1:
                    emit_attn(c - 1, hh, h_loc)
                emit_attn(c, 0, hh)
              emit_attn(n_tc - 1, hh, h_loc)
              for qc in range(n_tc):
                  emit_p3(qc, evict_eng=nc.vector if qc == n_tc - 1
                          else None)
              if dbg:
                nc.sync.dma_start(out=v_dump[:, :], in_=v_sb[:, :])
                for ft in range(n_ft):
                    nc.sync.dma_start(
                        out=qh_dump[ft * 128:(ft + 1) * 128, :],
                        in_=qhs[ft][:, :])
                    nc.sync.dma_start(
                        out=kh_dump[ft * 128:(ft + 1) * 128, :],
                        in_=khs[ft][:, :])
                    nc.sync.dma_start(
                        out=ot_dump[ft * 128:(ft + 1) * 128, :],
                        in_=ot_sb[:, ft * T:(ft + 1) * T])

    nc.compile()
    return nc


# ---------------- host-side preparation ----------------

def _rope_tables(T, start_pos, heads, scale):
    """Reference RoPE uses a PER-HEAD angle: theta = base[head] * pos.
    Returns [len(heads)*64, T] fp32 ct / st tables in the permuted
    [evens|odds] feature order, times `scale` (fp8 dequant fold).
    st is TARGET-indexed: even-half rows carry -s, odd-half +s
    (qs[a] = ps[a^32] * st[a])."""
    base = 1.0 / (10000.0 ** (np.arange(0, HD, 2, dtype=np.float64) / HD))
    pos = start_pos + np.arange(T, dtype=np.float64)
    F_ = len(heads) * HD
    ct = np.empty((F_, T), np.float32)
    st = np.empty((F_, T), np.float32)
    for i, g in enumerate(heads):
        th = base[g] * pos                        # [T]
        # cos feeds the already-dequantized fp16 copy (no scale); sin
        # multiplies the raw scaled PSUM, so it carries the dequant fold
        c, s = np.cos(th), np.sin(th) * scale
        b = i * HD
        ct[b:b + 64] = c[None, :]
        st[b:b + 32] = -s[None, :]
        st[b + 32:b + 64] = s[None, :]
    return ct, st


def _perm():
    p = np.empty(HD, np.int64)
    p[:32] = np.arange(0, HD, 2)
    p[32:] = np.arange(1, HD, 2)
    return p


def _hi_lo_pack(a, scale):
    """a: [rows, cols] fp32 -> [rows, 2*cols] e4m3 (hi | lo per row)."""
    s = (a * scale).astype(np.float32)
    hi = s.astype(E4M3)
    lo = (s - hi.astype(np.float32)).astype(E4M3)
    out = np.empty((a.shape[0], 2 * a.shape[1]), E4M3)
    out[:, :a.shape[1]] = hi
    out[:, a.shape[1]:] = lo
    return out


def _classify_mask(mask, T, TC):
    """mask: [S,S] additive (rows=q, cols=k). Returns block_kind[kt][qc] and
    packed exp-mask slabs ([k,q] orientation) for 'mask' blocks."""
    n_kt = T // 128
    n_tc = T // TC
    tri = (np.arange(TC)[None, :] >= np.arange(128)[:, None])
    kinds = [[None] * n_tc for _ in range(n_kt)]
    for kt in range(n_kt):
        for qc in range(n_tc):
            blk = mask[qc * TC:(qc + 1) * TC, kt * 128:(kt + 1) * 128].T
            if np.all(blk <= -1e8):
                kinds[kt][qc] = "skip"
            elif np.all(blk == 0.0):
                kinds[kt][qc] = "full"
            else:
                off = kt * 128 - qc * TC
                is_tri = False
                if 0 <= off <= TC - 128:
                    ref = np.full((128, TC), -1e9, np.float32)
                    ref[:, off:] = np.where(tri[:, :TC - off], 0.0, -1e9)
                    is_tri = bool(np.array_equal(blk, ref))
                kinds[kt][qc] = "tri" if is_tri else "mask"
    # a 'tri' block may not open an accumulation group at off>0
    for qc in range(n_tc):
        for kt in range(n_kt):
            k = kinds[kt][qc]
            if k == "skip":
                continue
            if k == "tri" and kt * 128 - qc * TC > 0:
                kinds[kt][qc] = "mask"
            break
    slabs = []
    for kt in range(n_kt):
        for qc in range(n_tc):
            if kinds[kt][qc] == "mask":
                blk = mask[qc * TC:(qc + 1) * TC,
                           kt * 128:(kt + 1) * 128].T
                slabs.append(np.exp(blk.astype(np.float64)
                                    ).astype(np.float32))
    me = (np.concatenate(slabs, axis=1) if slabs
          else np.zeros((128, TC), np.float32))
    return kinds, me


_CACHE = {}


def get_nc(cfg: Cfg, block_kind):
    key = (cfg.key(), tuple(tuple(r) for r in block_kind))
    if key not in _CACHE:
        _CACHE[key] = build_nc(cfg, block_kind)
    return _CACHE[key]


def prepare_in_maps(x, wq, wk, wv, wo, mask, start_pos, cfg):
    x = np.asarray(x, np.float32)
    wq = np.asarray(wq, np.float32)
    wk = np.asarray(wk, np.float32)
    wv = np.asarray(wv, np.float32)
    wo = np.asarray(wo, np.float32)
    mask2d = np.asarray(mask, np.float32).reshape(mask.shape[-2],
                                                  mask.shape[-1])
    sp = int(np.asarray(start_pos))

    tri01 = (np.arange(128)[None, :] >= np.arange(128)[:, None]
             ).astype(np.float16)
    kinds, me = _classify_mask(mask2d, cfg.T, cfg.TC)

    perm = _perm()
    x8_b = [_hi_lo_pack(np.ascontiguousarray(x[b].T), SX)
            for b in range(BATCH)]
    # interleave hi/lo per row: [D, 2T] with row d = [hi(T) | lo(T)]
    in_maps = []
    for core in range(N_CORES):
        b = core // TP
        tp = core % TP
        heads = np.arange(tp * cfg.h_loc, (tp + 1) * cfg.h_loc)
        ct, st = _rope_tables(cfg.T, sp, heads, 1.0 / (SW * SX))
        cst = np.concatenate([ct, st], axis=1).astype(np.float16)
        rows = (heads[:, None] * HD + perm[None, :]).reshape(-1)
        rows_plain = (heads[:, None] * HD
                      + np.arange(HD)[None, :]).reshape(-1)
        in_maps.append({
            "x8T": x8_b[b],
            "wq8": _hi_lo_pack(np.ascontiguousarray(wq[rows, :].T), SW),
            "wk8": _hi_lo_pack(np.ascontiguousarray(wk[rows, :].T), SW),
            "wv8": _hi_lo_pack(np.ascontiguousarray(wv[rows_plain, :].T),
                               SW),
            "woT": np.ascontiguousarray(wo[:, rows_plain].T
                                        ).astype(np.float16),
            "cst": cst, "tri": tri01,
            "maskexp": me.astype(np.float16),
            "ones": np.ones((128, 1), np.float16),
            "nbias": np.full((128, 1), EXP_BIAS, np.float32),
        })
    return in_maps, kinds


def kernel(x, wq, wk, wv, wo, mask, start_pos):
    cfg = Cfg()
    in_maps, kinds = prepare_in_maps(x, wq, wk, wv, wo, mask, start_pos, cfg)
    nc = get_nc(cfg, kinds)
    out = run_bass_kernel_spmd(nc, in_maps, core_ids=list(range(N_CORES)))
    y = np.zeros((BATCH, SEQ, DIM), np.float32)
    for core in range(N_CORES):
        y[core // TP] += out.results[core]["y"].astype(np.float32)
    return y
